# revision 10
# baseline (speedup 1.0000x reference)
"""Bass/Trainium2 kernel for nn_Differential_Attention_60825326846200.

Mathematical reduction of the reference:
  scores[b,h,i,j] = (sum_d q[b,h,i,d] - k[b,h,i,d]) / sqrt(DH) + mask[b,i]
is constant over the key index j, so the softmax over j is exactly the
uniform distribution (1/S) regardless of q, k, and the mask.  Hence
  ctx[b,h,i,:] = mean_j v[b,h,j,:]          (independent of i)
  out[b,i,:]   = (mean_j hidden_b[b,j,:]) @ Wv.T + bv   for every i.
The q/k projections and the attention mask cancel exactly, and the output
is rank-1 along the sequence axis: 2048 identical rows per batch.

ONE SPMD launch, contraction-sharded (no cross-core exchange needed):
core c owns HID columns d in [128c, 128c+128).

  Because the hidden dim (not the sequence) is sharded, each core's
  sequence reduction is COMPLETE for its slice: it reads
  hidden_b[:, :, d_c] (2MB), reduces over all 2048 positions on the PE
  (data-stationary matmuls against a ones column -> m[d, b] lands in
  PSUM already transposed for the next step), then contracts its 128
  columns with its Wv slice (wvt[d, o] = Wv[o, d_c], 512KB) ->
  z_c[b, o] = sum_{d in c} m[d, b] * Wv[o, d], a contraction-partial of
  the unique output row.  Core 0's bias input carries S*bv (others
  zeros), added via a rank-1 matmul into the same PSUM accumulation;
  ACT/DVE evacuate the two 512-col halves with an exact 1/S scale.

  Host unshard = the standard gather for contraction sharding: sum the
  8 partials [2, 1024] and broadcast over the sequence axis (the output
  is rank-1: every row within a batch is the same vector).

  Per-core HW traffic: 2.52MB in, 8KB out (vs 4.2MB in / 2MB out for a
  seq-sharded two-launch version), and one launch's fixed
  prolog/epilog (~14us) instead of two.
"""

import numpy as np

import concourse.bacc as bacc
import concourse.mybir as mybir
import concourse.tile as tile
from concourse.bass_utils import run_bass_kernel_spmd

N_CORES = 8
B, S, HID = 2, 2048, 1024
D_LOC = HID // N_CORES  # 128 hidden columns owned per core
NBLK = S // 128  # 16 seq blocks of 128 positions
NCHUNK = 4  # input DMA chunks (512 seq positions each)
F32 = mybir.dt.float32
F32R = mybir.dt.float32r
BF16 = mybir.dt.bfloat16

_compiled = None


def _new_nc():
    return bacc.Bacc(
        "TRN2",
        target_bir_lowering=False,
        debug=False,
        enable_asserts=False,
        num_devices=N_CORES,
    )


def _build():
    """Single launch: complete seq-reduction of this core's column slice,
    projection through its Wv rows, contraction-partial out.
    Inputs:
      "hbt" [128, NBLK*B*128]: hbt[d, ((blk*B)+b)*128 + p] is NOT the
        layout -- see below.  Partition dim is the seq position within a
        block: hbt[p, (blk, b, d)] = hb[b, blk*128+p, 128*core+d].
      "wvt" [128, HID]: wvt[d, o] = Wv[o, 128*core+d]
      "bvS" [1, HID]: S*bv on core 0, zeros elsewhere
    Output "zout" [B, HID]: zout[b, o] = (sum_{d in slice} mbar[b, d] *
      Wv[o, d]) + bv[o]*(core==0), where mbar is the full-sequence mean."""
    nc = _new_nc()
    hbt = nc.dram_tensor(
        "hbt", [128, NCHUNK, B, S // NCHUNK], F32R, kind="ExternalInput"
    ).ap()
    wvt = nc.dram_tensor("wvt", [128, HID], F32R, kind="ExternalInput").ap()
    bvS = nc.dram_tensor("bvS", [1, HID], F32R, kind="ExternalInput").ap()
    consts = nc.dram_tensor("consts", [1, 2], F32R, kind="ExternalInput").ap()
    zout = nc.dram_tensor("zout", [B, HID], F32, kind="ExternalOutput").ap()

    scs = S // NCHUNK  # seq positions per input chunk

    with tile.TileContext(nc) as tc:
        with (
            tc.tile_pool(name="big", bufs=1) as big,
            tc.tile_pool(name="small", bufs=1) as small,
            tc.tile_pool(name="psum", bufs=1, space="PSUM") as psum,
        ):
            hb_sb = big.tile([128, NCHUNK, B, scs], F32R)
            wvt_sb = big.tile([128, HID], F32R)
            bvS_sb = small.tile([1, HID], F32R)
            consts_sb = small.tile([1, 2], F32R)
            # Three descriptor-generation paths in parallel (each HWDGE
            # ring reads at only ~135 GB/s): sync ring c0 + c3_lo + wvt0,
            # scalar (ACT) ring c1 + c3_hi + wvt1, gpsimd SWDGE the tiny
            # loads + c2.  wvt goes last on its ring: it gates only the
            # final projection, which also needs m (ready ~1us after the
            # last hbt chunk).  c3 is split by partition halves so both
            # HWDGE rings finish their hbt share together.
            nc.gpsimd.dma_start(consts_sb[:], consts[:])
            nc.gpsimd.dma_start(bvS_sb[:], bvS[:])
            nc.sync.dma_start(hb_sb[:, 0], hbt[:, 0])
            nc.scalar.dma_start(hb_sb[:, 1], hbt[:, 1])
            nc.gpsimd.dma_start(hb_sb[:, 2], hbt[:, 2])
            nc.sync.dma_start(hb_sb[0:64, 3], hbt[0:64, 3])
            nc.scalar.dma_start(hb_sb[64:128, 3], hbt[64:128, 3])
            nc.sync.dma_start(wvt_sb[:, 0:512], wvt[:, 0:512])
            nc.scalar.dma_start(wvt_sb[:, 512:1024], wvt[:, 512:1024])

            ones2 = consts_sb[0:1, 0:2]
            scratch = small.tile([128, 512], BF16)
            nc.vector.memset(scratch[:], 1.0)
            # PE HAM clock-gate warmup while the first DMAs stream
            pw = psum.tile([2, 512], F32, name="pwarm", tag="pwarm")
            for _ in range(8):
                nc.tensor.matmul(
                    pw[:], lhsT=scratch[:, 0:2], rhs=scratch[:], start=True, stop=True
                )

            # seq reduction: one [128, 512] free-axis reduce per (chunk,
            # batch) on the DVE as each chunk lands (free-axis
            # tensor_reduce is DVE-only).  Partitions are this core's 128
            # hidden columns, so the result m[d, b] is already transposed
            # for the projection lhsT.  No ACT ops anywhere: the first ACT
            # op would emit a ~1.3us ACT_TABLE_LOAD on the scalar
            # sequencer, ahead of -- and delaying -- the scalar ring's DMA
            # issues.
            mparts = small.tile([128, NCHUNK, B], F32)
            with nc.allow_low_precision(reason="float32r is fp32-width"):
                for k in range(NCHUNK):
                    nc.vector.reduce_sum(
                        mparts[:, k, 0:1], hb_sb[:, k, 0], axis=mybir.AxisListType.X
                    )
                    nc.vector.reduce_sum(
                        mparts[:, k, 1:2], hb_sb[:, k, 1], axis=mybir.AxisListType.X
                    )
            m_sb = small.tile([128, B], F32R)
            with nc.allow_low_precision(reason="float32r is fp32-width"):
                nc.vector.reduce_sum(
                    m_sb[:],
                    mparts[:].rearrange("p k b -> p b k"),
                    axis=mybir.AxisListType.X,
                )
                # fold the exact 1/S mean scaling into the tiny m operand
                # (128 partitions x 2) instead of the [2, 1024] z evac
                nc.vector.tensor_scalar_mul(m_sb[:], m_sb[:], 1.0 / S)

            # projection: z[b, o] = sum_d m[d, b]*wvt[d, o] (+ S*bv[o] via
            # the rank-1 bias matmul, which starts each accumulation group
            # -- it depends only on the small early loads)
            z_sb = small.tile([B, HID], F32)
            for h, weng in ((0, nc.sync), (1, nc.scalar)):
                zp = psum.tile([B, 512], F32, name=f"z{h}", tag=f"z{h}")
                nc.tensor.matmul(
                    zp[:],
                    lhsT=ones2,
                    rhs=bvS_sb[:, h * 512 : (h + 1) * 512],
                    start=True,
                    stop=False,
                )
                nc.tensor.matmul(
                    zp[:],
                    lhsT=m_sb[:],
                    rhs=wvt_sb[:, h * 512 : (h + 1) * 512],
                    start=False,
                    stop=True,
                )
                # plain evac (m carried the 1/S, bias input is plain bv);
                # each half's write leaves on its own idle HWDGE ring as
                # soon as its evac lands
                nc.vector.tensor_copy(z_sb[:, h * 512 : (h + 1) * 512], zp[:])
                weng.dma_start(
                    zout[:, h * 512 : (h + 1) * 512], z_sb[:, h * 512 : (h + 1) * 512]
                )
    nc.compile()
    return nc


def get_nc():
    global _compiled
    if _compiled is None:
        _compiled = _build()
    return _compiled


def make_in_maps(inputs):
    hb = np.asarray(inputs["hidden_states_b"], dtype=np.float32)
    Wv = np.asarray(inputs["Wv"], dtype=np.float32)
    bv = np.asarray(inputs["bv"], dtype=np.float32)
    consts = np.ones((1, 2), dtype=np.float32)
    bvS = np.zeros((N_CORES, 1, HID), dtype=np.float32)
    bvS[0, 0] = bv
    scs = S // NCHUNK
    maps = []
    for c in range(N_CORES):
        sl = hb[:, :, c * D_LOC : (c + 1) * D_LOC]  # [B, S, 128]
        # hbt[d, k, b, s] = hb[b, k*scs+s, c*128+d]
        t = sl.reshape(B, NCHUNK, scs, D_LOC).transpose(3, 1, 0, 2)
        wt = Wv[:, c * D_LOC : (c + 1) * D_LOC].T  # [128 d, HID o]
        maps.append(
            {
                "hbt": np.ascontiguousarray(t),
                "wvt": np.ascontiguousarray(wt),
                "bvS": bvS[c],
                "consts": consts,
            }
        )
    return maps


def combine(results):
    # unshard for contraction sharding: sum the 8 partials (bias was
    # folded into core 0's partial, 1/S scaling done on-device), then
    # broadcast the unique per-batch row over the sequence axis
    z = results[0]["zout"].copy()
    for c in range(1, N_CORES):
        z += results[c]["zout"]
    return np.ascontiguousarray(np.broadcast_to(z[:, None, :], (B, S, HID)))


def kernel(**inputs) -> np.ndarray:
    nc = get_nc()
    res = run_bass_kernel_spmd(nc, make_in_maps(inputs), list(range(N_CORES)))
    return combine(res.results)


# revision 11
# speedup vs baseline: 1.0535x; 1.0535x over previous
"""Bass/Trainium2 kernel for nn_Differential_Attention_60825326846200.

Mathematical reduction of the reference:
  scores[b,h,i,j] = (sum_d q[b,h,i,d] - k[b,h,i,d]) / sqrt(DH) + mask[b,i]
is constant over the key index j, so the softmax over j is exactly the
uniform distribution (1/S) regardless of q, k, and the mask.  Hence
  ctx[b,h,i,:] = mean_j v[b,h,j,:]          (independent of i)
  out[b,i,:]   = (mean_j hidden_b[b,j,:]) @ Wv.T + bv   for every i.
The q/k projections and the attention mask cancel exactly, and the output
is rank-1 along the sequence axis: 2048 identical rows per batch.

ONE SPMD launch, contraction-sharded (no cross-core exchange needed):
core c owns HID columns d in [128c, 128c+128).

  Because the hidden dim (not the sequence) is sharded, each core's
  sequence reduction is COMPLETE for its slice: it reads
  hidden_b[:, :, d_c], reduces over all 2048 positions on the DVE
  (partitions = the 128 hidden columns, so m[d, b] lands already
  transposed for the projection lhsT), then contracts its 128 columns
  with its Wv slice -> z_c[b, o], a contraction-partial of the unique
  output row.  Core 0's bias input carries bv (others zeros), added via
  a rank-1 PE matmul into the same PSUM accumulation; m is pre-scaled
  by the exact 1/S so the PSUM holds final values and the evacuation is
  a plain copy.

  Host unshard = the standard gather for contraction sharding: sum the
  8 partials [2, 1024] and broadcast over the sequence axis (the output
  is rank-1: every row within a batch is the same vector).

  The inputs stream in bf16 (cast on the host while laying out the
  shards -- the 2e-2 tolerance is far above bf16's ~5e-3 here, and the
  f32 baseline already ran its matmuls in TF32-width float32r): 1.26MB
  in / 8KB out per core.  HW lessons baked in: HWDGE read bandwidth is
  ~130GB/s per ring on 4KB runs and the two rings share the 16 SDMA
  engines (~260GB/s aggregate); SWDGE (gpsimd) is ~50GB/s -- good only
  for tiny/small loads; partition-sliced DMAs use only half the SDMA
  engines (never split below 128 partitions); the first ACT op would
  emit a ~1.3us ACT_TABLE_LOAD ahead of the scalar ring's DMA issues,
  so no ACT ops are used at all.
"""

import numpy as np
import ml_dtypes

import concourse.bacc as bacc
import concourse.mybir as mybir
import concourse.tile as tile
from concourse.bass_utils import run_bass_kernel_spmd

N_CORES = 8
B, S, HID = 2, 2048, 1024
D_LOC = HID // N_CORES  # 128 hidden columns owned per core
# uneven seq chunks, issued so the DVE reduce of each chunk hides under
# the arrival of the next and m is ready just as wvt lands
CHUNKS = (1024, 512, 256, 256)
COFF = [0]
for _scs in CHUNKS:
    COFF.append(COFF[-1] + B * _scs)  # flat (k, b, s) col offsets
F32 = mybir.dt.float32
F32R = mybir.dt.float32r
BF16 = mybir.dt.bfloat16
NPBF16 = ml_dtypes.bfloat16

_compiled = None


def _new_nc():
    return bacc.Bacc(
        "TRN2",
        target_bir_lowering=False,
        debug=False,
        enable_asserts=False,
        num_devices=N_CORES,
    )


def _build():
    """Single launch: complete seq-reduction of this core's column slice,
    projection through its Wv rows, contraction-partial out.
    Inputs:
      "hbt" [128, sum(2*scs)] bf16, chunk-major flat cols (k, b, s):
        hbt[d, (k, b, s)] = hb[b, chunk_k_start + s, 128*core + d]
      "wvt" [128, HID] bf16: wvt[d, o] = Wv[o, 128*core+d]
      "bvS" [1, HID] bf16: bv on core 0, zeros elsewhere
      "consts" [1, 2] bf16: ones (bias-matmul stationary)
    Output "zout" [B, HID] f32: this core's contraction-partial of the
    unique output row (bias included on core 0, 1/S applied)."""
    nc = _new_nc()
    hbt = nc.dram_tensor("hbt", [128, COFF[-1]], BF16, kind="ExternalInput").ap()
    wvt = nc.dram_tensor("wvt", [128, HID], BF16, kind="ExternalInput").ap()
    bvS = nc.dram_tensor("bvS", [1, HID], BF16, kind="ExternalInput").ap()
    consts = nc.dram_tensor("consts", [1, 2], BF16, kind="ExternalInput").ap()
    zout = nc.dram_tensor("zout", [B, HID], F32, kind="ExternalOutput").ap()

    with tile.TileContext(nc) as tc:
        with (
            tc.tile_pool(name="big", bufs=1) as big,
            tc.tile_pool(name="small", bufs=1) as small,
            tc.tile_pool(name="psum", bufs=1, space="PSUM") as psum,
        ):
            hb_sb = big.tile([128, COFF[-1]], BF16)
            wvt_sb = big.tile([128, HID], BF16)
            bvS_sb = small.tile([1, HID], BF16)
            consts_sb = small.tile([1, 2], BF16)

            def hb_dma(eng, k):
                eng.dma_start(
                    hb_sb[:, COFF[k] : COFF[k + 1]], hbt[:, COFF[k] : COFF[k + 1]]
                )

            # gpsimd SWDGE takes the tiny loads + the smallest chunk (~50
            # GB/s, but off the HWDGE rings); the two HWDGE rings carry
            # ~640KB each, wvt last (it gates only the final projection)
            nc.gpsimd.dma_start(consts_sb[:], consts[:])
            nc.gpsimd.dma_start(bvS_sb[:], bvS[:])
            hb_dma(nc.scalar, 1)  # 256KB, lands first on its ring
            hb_dma(nc.sync, 0)  # 512KB
            hb_dma(nc.gpsimd, 3)  # 128KB via SWDGE
            hb_dma(nc.sync, 2)  # 128KB
            nc.scalar.dma_start(wvt_sb[:], wvt[:])  # 256KB

            ones2 = consts_sb[0:1, 0:2]
            scratch = small.tile([128, 512], BF16)
            nc.vector.memset(scratch[:], 1.0)
            # PE HAM clock-gate warmup while the first DMAs stream
            pw = psum.tile([2, 512], F32, name="pwarm", tag="pwarm")
            for _ in range(6):
                nc.tensor.matmul(
                    pw[:], lhsT=scratch[:, 0:2], rhs=scratch[:], start=True, stop=True
                )

            # seq reduction: one [128, scs] free-axis DVE reduce per
            # (chunk, batch), issued in expected arrival order.
            mparts = small.tile([128, len(CHUNKS) * B], F32)
            with nc.allow_low_precision(reason="bf16 in, f32 accumulate"):
                for k in (1, 3, 0, 2):
                    for b2 in range(B):
                        nc.vector.reduce_sum(
                            mparts[:, k * B + b2 : k * B + b2 + 1],
                            hb_sb[:, COFF[k] + b2 * CHUNKS[k] :][:, : CHUNKS[k]],
                            axis=mybir.AxisListType.X,
                        )
            # keep the PE clock ungated between the warmups and the
            # projection: two matmuls fed by the last sync-ring chunk
            for _ in range(2):
                nc.tensor.matmul(
                    pw[:],
                    lhsT=hb_sb[:, COFF[2] : COFF[2] + 2],
                    rhs=hb_sb[:, COFF[2] : COFF[2] + 512],
                    start=True,
                    stop=True,
                )
            # combine the chunk partials and fold in the exact 1/S mean
            # scaling (tiny: 128 partitions x 2 outputs); bf16 output is
            # the projection's stationary operand
            mtmp = small.tile([128, B], F32)
            m_sb = small.tile([128, B], BF16)
            with nc.allow_low_precision(reason="bf16 operand for bf16 matmul"):
                nc.vector.reduce_sum(
                    mtmp[:],
                    mparts[:].rearrange("p (k b) -> p b k", b=B),
                    axis=mybir.AxisListType.X,
                )
                nc.vector.tensor_scalar_mul(m_sb[:], mtmp[:], 1.0 / S)

            # projection: z[b, o] = sum_d m[d, b]*wvt[d, o] + bv[o]; the
            # rank-1 bias matmul starts each accumulation group (it
            # depends only on the tiny early loads)
            z_sb = small.tile([B, HID], F32)
            for h, weng in ((0, nc.sync), (1, nc.scalar)):
                zp = psum.tile([B, 512], F32, name=f"z{h}", tag=f"z{h}")
                nc.tensor.matmul(
                    zp[:],
                    lhsT=ones2,
                    rhs=bvS_sb[:, h * 512 : (h + 1) * 512],
                    start=True,
                    stop=False,
                )
                nc.tensor.matmul(
                    zp[:],
                    lhsT=m_sb[:],
                    rhs=wvt_sb[:, h * 512 : (h + 1) * 512],
                    start=False,
                    stop=True,
                )
                # plain evac (m carried the 1/S, bias is in the PSUM);
                # each half's write leaves on its own HWDGE ring as soon
                # as its evac lands
                nc.vector.tensor_copy(z_sb[:, h * 512 : (h + 1) * 512], zp[:])
                weng.dma_start(
                    zout[:, h * 512 : (h + 1) * 512], z_sb[:, h * 512 : (h + 1) * 512]
                )
    nc.compile()
    return nc


def get_nc():
    global _compiled
    if _compiled is None:
        _compiled = _build()
    return _compiled


def make_in_maps(inputs):
    hb = np.asarray(inputs["hidden_states_b"], dtype=np.float32)
    Wv = np.asarray(inputs["Wv"], dtype=np.float32)
    bv = np.asarray(inputs["bv"], dtype=np.float32)
    consts = np.ones((1, 2), dtype=NPBF16)
    bvS = np.zeros((N_CORES, 1, HID), dtype=NPBF16)
    bvS[0, 0] = bv.astype(NPBF16)
    maps = []
    for c in range(N_CORES):
        sl = hb[:, :, c * D_LOC : (c + 1) * D_LOC].astype(NPBF16)  # [B, S, 128]
        # chunk-major flat layout: block k is [128, B, scs_k]
        blocks = []
        s0 = 0
        for scs in CHUNKS:
            blk = sl[:, s0 : s0 + scs, :].transpose(2, 0, 1)  # [128, B, scs]
            blocks.append(blk.reshape(128, B * scs))
            s0 += scs
        wt = Wv[:, c * D_LOC : (c + 1) * D_LOC].T.astype(NPBF16)  # [128 d, HID o]
        maps.append(
            {
                "hbt": np.ascontiguousarray(np.concatenate(blocks, axis=1)),
                "wvt": np.ascontiguousarray(wt),
                "bvS": bvS[c],
                "consts": consts,
            }
        )
    return maps


def combine(results):
    # unshard for contraction sharding: sum the 8 partials (bias was
    # folded into core 0's partial, 1/S scaling done on-device), then
    # broadcast the unique per-batch row over the sequence axis
    z = results[0]["zout"].copy()
    for c in range(1, N_CORES):
        z += results[c]["zout"]
    return np.ascontiguousarray(np.broadcast_to(z[:, None, :], (B, S, HID)))


def kernel(**inputs) -> np.ndarray:
    nc = get_nc()
    res = run_bass_kernel_spmd(nc, make_in_maps(inputs), list(range(N_CORES)))
    return combine(res.results)


# revision 14
# speedup vs baseline: 1.2681x; 1.2037x over previous
"""Bass/Trainium2 kernel for nn_Differential_Attention_60825326846200.

Mathematical reduction of the reference:
  scores[b,h,i,j] = (sum_d q[b,h,i,d] - k[b,h,i,d]) / sqrt(DH) + mask[b,i]
is constant over the key index j, so the softmax over j is exactly the
uniform distribution (1/S) regardless of q, k, and the mask.  Hence
  ctx[b,h,i,:] = mean_j v[b,h,j,:]          (independent of i)
  out[b,i,:]   = (mean_j hidden_b[b,j,:]) @ Wv.T + bv   for every i.
The q/k projections and the attention mask cancel exactly, and the output
is rank-1 along the sequence axis: 2048 identical rows per batch.

ONE SPMD launch, contraction-sharded (no cross-core exchange needed):
core c owns HID columns d in [128c, 128c+128).

  Because the hidden dim (not the sequence) is sharded, each core's
  sequence reduction is COMPLETE for its slice: it reads
  hidden_b[:, :, d_c], reduces over all 2048 positions on the DVE
  (partitions = the 128 hidden columns, so m[d, b] lands already
  transposed for the projection lhsT), then contracts its 128 columns
  with its Wv slice -> z_c[b, o], a contraction-partial of the unique
  output row.  Core 0's bias input carries bv (others zeros), added via
  a rank-1 PE matmul into the same PSUM accumulation; m is pre-scaled
  by the exact 1/S so the PSUM holds final values and the evacuation is
  a plain copy.

  Host unshard = the standard gather for contraction sharding: sum the
  8 partials [2, 1024] and broadcast over the sequence axis (the output
  is rank-1: every row within a batch is the same vector).

  The inputs stream in bf16 (cast on the host while laying out the
  shards -- the 2e-2 tolerance is far above bf16's ~5e-3 here, and the
  f32 baseline already ran its matmuls in TF32-width float32r): 1.26MB
  in / 8KB out per core.  HW lessons baked in: HWDGE read bandwidth is
  ~130GB/s per ring on 4KB runs and the two rings share the 16 SDMA
  engines (~260GB/s aggregate); SWDGE (gpsimd) is ~50GB/s -- good only
  for tiny/small loads; partition-sliced DMAs use only half the SDMA
  engines (never split below 128 partitions); the first ACT op would
  emit a ~1.3us ACT_TABLE_LOAD ahead of the scalar ring's DMA issues,
  so no ACT ops are used at all.
"""

import numpy as np
import ml_dtypes

import concourse.bacc as bacc
import concourse.mybir as mybir
import concourse.tile as tile
from concourse.bass_utils import run_bass_kernel_spmd

N_CORES = 8
B, S, HID = 2, 2048, 1024
D_LOC = HID // N_CORES  # 128 hidden columns owned per core
SH = S // 2  # seq halves: 4 stream pieces (batch x half), reduced as they land
F32 = mybir.dt.float32
F32R = mybir.dt.float32r
BF16 = mybir.dt.bfloat16
NPBF16 = ml_dtypes.bfloat16

_compiled = None


def _new_nc():
    return bacc.Bacc(
        "TRN2",
        target_bir_lowering=False,
        debug=False,
        enable_asserts=False,
        num_devices=N_CORES,
    )


def _build():
    """Single launch: complete seq-reduction of this core's column slice,
    projection through its Wv rows, contraction-partial out.
    Inputs:
      "hbt" [128, sum(2*scs)] bf16, chunk-major flat cols (k, b, s):
        hbt[d, (k, b, s)] = hb[b, chunk_k_start + s, 128*core + d]
      "wvt" [128, HID] bf16: wvt[d, o] = Wv[o, 128*core+d]
      "bvS" [1, HID] bf16: bv on core 0, zeros elsewhere
      "consts" [1, 2] bf16: ones (bias-matmul stationary)
    Output "zout" [B, HID] f32: this core's contraction-partial of the
    unique output row (bias included on core 0, 1/S applied)."""
    nc = _new_nc()
    hbt = nc.dram_tensor("hbt", [128, B, 2, SH], BF16, kind="ExternalInput").ap()
    wvt = nc.dram_tensor("wvt", [128, HID], BF16, kind="ExternalInput").ap()
    # bvS[0, 0:2] = ones (bias-matmul stationary), bvS[0, 2:] = bv or 0
    bvS = nc.dram_tensor("bvS", [1, 2 + HID], BF16, kind="ExternalInput").ap()
    zout = nc.dram_tensor("zout", [B, HID], F32, kind="ExternalOutput").ap()

    with tile.TileContext(nc) as tc:
        with (
            tc.tile_pool(name="big", bufs=1) as big,
            tc.tile_pool(name="small", bufs=1) as small,
            tc.tile_pool(name="psum", bufs=1, space="PSUM") as psum,
        ):
            hb_sb = big.tile([128, B, 2, SH], BF16)
            wvt_sb = big.tile([128, HID], BF16)
            bvS_sb = small.tile([1, 2 + HID], BF16)

            # Four 256KB pieces (batch x seq-half), two per HWDGE ring,
            # reduced by the DVE in arrival order.  The tiny bias row
            # rides SWDGE (off the HWDGE rings); wvt goes last on the
            # sync ring -- it gates only the final projection, which also
            # needs m (ready ~1.5us after the last hbt piece).
            nc.gpsimd.dma_start(bvS_sb[:], bvS[:])
            nc.sync.dma_start(hb_sb[:, 0, 0], hbt[:, 0, 0])
            nc.scalar.dma_start(hb_sb[:, 1, 0], hbt[:, 1, 0])
            nc.sync.dma_start(hb_sb[:, 0, 1], hbt[:, 0, 1])
            nc.scalar.dma_start(hb_sb[:, 1, 1], hbt[:, 1, 1])
            nc.sync.dma_start(wvt_sb[:], wvt[:])

            ones2 = bvS_sb[0:1, 0:2]
            scratch = small.tile([128, 512], BF16)
            nc.vector.memset(scratch[:], 1.0)
            # PE and DVE clock-gate warmups while the first DMAs stream
            pw = psum.tile([2, 512], F32, name="pwarm", tag="pwarm")
            for _ in range(6):
                nc.tensor.matmul(
                    pw[:], lhsT=scratch[:, 0:2], rhs=scratch[:], start=True, stop=True
                )
            for _ in range(3):
                nc.vector.tensor_copy(scratch[:], scratch[:])

            # seq reduction: one [128, SH] free-axis DVE reduce per
            # (batch, half) in arrival order, a tiny combine, then the
            # exact 1/S mean scaling into the projection's bf16 stationary
            mtmp = small.tile([128, B * 2], F32)
            mcmb = small.tile([128, B], F32)
            m_sb = small.tile([128, B], BF16)
            with nc.allow_low_precision(reason="bf16 in, f32 accumulate"):
                for half in range(2):
                    for b2 in range(B):
                        nc.vector.reduce_sum(
                            mtmp[:, b2 * 2 + half : b2 * 2 + half + 1],
                            hb_sb[:, b2, half],
                            axis=mybir.AxisListType.X,
                        )
                nc.vector.reduce_sum(
                    mcmb[:],
                    mtmp[:].rearrange("p (b h) -> p b h", b=B),
                    axis=mybir.AxisListType.X,
                )
                nc.vector.tensor_scalar_mul(m_sb[:], mcmb[:], 1.0 / S)
            # keep the PE clock ungated between the warmups and the
            # projection, fed by batch-0's first piece
            for _ in range(2):
                nc.tensor.matmul(
                    pw[:],
                    lhsT=hb_sb[:, 0, 0, 0:2],
                    rhs=hb_sb[:, 0, 0, 0:512],
                    start=True,
                    stop=True,
                )

            # projection: z[b, o] = sum_d m[d, b]*wvt[d, o] + bv[o]; the
            # rank-1 bias matmul starts each accumulation group (it
            # depends only on the tiny early loads)
            z_sb = small.tile([B, HID], F32)
            for h, weng in ((0, nc.sync), (1, nc.scalar)):
                zp = psum.tile([B, 512], F32, name=f"z{h}", tag=f"z{h}")
                nc.tensor.matmul(
                    zp[:],
                    lhsT=ones2,
                    rhs=bvS_sb[:, 2 + h * 512 : 2 + (h + 1) * 512],
                    start=True,
                    stop=False,
                )
                nc.tensor.matmul(
                    zp[:],
                    lhsT=m_sb[:],
                    rhs=wvt_sb[:, h * 512 : (h + 1) * 512],
                    start=False,
                    stop=True,
                )
                # plain evac (m carried the 1/S, bias is in the PSUM);
                # each half's write leaves on its own HWDGE ring as soon
                # as its evac lands
                nc.vector.tensor_copy(z_sb[:, h * 512 : (h + 1) * 512], zp[:])
                weng.dma_start(
                    zout[:, h * 512 : (h + 1) * 512], z_sb[:, h * 512 : (h + 1) * 512]
                )
    nc.compile()
    return nc


def get_nc():
    global _compiled
    if _compiled is None:
        _compiled = _build()
    return _compiled


def make_in_maps(inputs):
    hb = np.asarray(inputs["hidden_states_b"], dtype=np.float32)
    Wv = np.asarray(inputs["Wv"], dtype=np.float32)
    bv = np.asarray(inputs["bv"], dtype=np.float32)
    bvS = np.zeros((N_CORES, 1, 2 + HID), dtype=NPBF16)
    bvS[:, 0, 0:2] = 1  # bias-matmul stationary ones
    bvS[0, 0, 2:] = bv.astype(NPBF16)
    maps = []
    for c in range(N_CORES):
        sl = hb[:, :, c * D_LOC : (c + 1) * D_LOC].astype(NPBF16)  # [B, S, 128]
        # hbt[d, b, half, s] = hb[b, half*SH + s, c*128 + d]
        t = sl.reshape(B, 2, SH, D_LOC).transpose(3, 0, 1, 2)
        wt = Wv[:, c * D_LOC : (c + 1) * D_LOC].T.astype(NPBF16)  # [128 d, HID o]
        maps.append(
            {
                "hbt": np.ascontiguousarray(t),
                "wvt": np.ascontiguousarray(wt),
                "bvS": bvS[c],
            }
        )
    return maps


def combine(results):
    # unshard for contraction sharding: sum the 8 partials (bias was
    # folded into core 0's partial, 1/S scaling done on-device), then
    # broadcast the unique per-batch row over the sequence axis
    z = results[0]["zout"].copy()
    for c in range(1, N_CORES):
        z += results[c]["zout"]
    return np.ascontiguousarray(np.broadcast_to(z[:, None, :], (B, S, HID)))


def kernel(**inputs) -> np.ndarray:
    nc = get_nc()
    res = run_bass_kernel_spmd(nc, make_in_maps(inputs), list(range(N_CORES)))
    return combine(res.results)


# revision 15
# speedup vs baseline: 1.2933x; 1.0199x over previous
"""Bass/Trainium2 kernel for nn_Differential_Attention_60825326846200.

Mathematical reduction of the reference:
  scores[b,h,i,j] = (sum_d q[b,h,i,d] - k[b,h,i,d]) / sqrt(DH) + mask[b,i]
is constant over the key index j, so the softmax over j is exactly the
uniform distribution (1/S) regardless of q, k, and the mask.  Hence
  ctx[b,h,i,:] = mean_j v[b,h,j,:]          (independent of i)
  out[b,i,:]   = (mean_j hidden_b[b,j,:]) @ Wv.T + bv   for every i.
The q/k projections and the attention mask cancel exactly, and the output
is rank-1 along the sequence axis: 2048 identical rows per batch.

ONE SPMD launch, contraction-sharded (no cross-core exchange needed):
core c owns HID columns d in [128c, 128c+128).

  Because the hidden dim (not the sequence) is sharded, each core's
  sequence reduction is COMPLETE for its slice: it reads
  hidden_b[:, :, d_c], reduces over all 2048 positions on the DVE
  (partitions = the 128 hidden columns, so m[d, b] lands already
  transposed for the projection lhsT), then contracts its 128 columns
  with its Wv slice -> z_c[b, o], a contraction-partial of the unique
  output row.  Core 0's bias input carries bv (others zeros), added via
  a rank-1 PE matmul into the same PSUM accumulation; m is pre-scaled
  by the exact 1/S so the PSUM holds final values and the evacuation is
  a plain copy.

  Host unshard = the standard gather for contraction sharding: sum the
  8 partials [2, 1024] and broadcast over the sequence axis (the output
  is rank-1: every row within a batch is the same vector).

  The inputs stream in bf16 (cast on the host while laying out the
  shards -- the 2e-2 tolerance is far above bf16's ~5e-3 here, and the
  f32 baseline already ran its matmuls in TF32-width float32r): 1.26MB
  in / 8KB out per core.  HW lessons baked in: HWDGE read bandwidth is
  ~130GB/s per ring on 4KB runs and the two rings share the 16 SDMA
  engines (~260GB/s aggregate); SWDGE (gpsimd) is ~50GB/s -- good only
  for tiny/small loads; partition-sliced DMAs use only half the SDMA
  engines (never split below 128 partitions); the first ACT op would
  emit a ~1.3us ACT_TABLE_LOAD ahead of the scalar ring's DMA issues,
  so no ACT ops are used at all.
"""

import numpy as np
import ml_dtypes

import concourse.bacc as bacc
import concourse.mybir as mybir
import concourse.tile as tile
from concourse.bass_utils import run_bass_kernel_spmd

N_CORES = 8
B, S, HID = 2, 2048, 1024
D_LOC = HID // N_CORES  # 128 hidden columns owned per core
SH = S // 2  # seq halves: 4 stream pieces (batch x half), reduced as they land
F32 = mybir.dt.float32
F32R = mybir.dt.float32r
BF16 = mybir.dt.bfloat16
NPBF16 = ml_dtypes.bfloat16

_compiled = None


def _new_nc():
    return bacc.Bacc(
        "TRN2",
        target_bir_lowering=False,
        debug=False,
        enable_asserts=False,
        num_devices=N_CORES,
    )


def _build():
    """Single launch: complete seq-reduction of this core's column slice,
    projection through its Wv rows, contraction-partial out.
    Inputs:
      "hbt" [128, sum(2*scs)] bf16, chunk-major flat cols (k, b, s):
        hbt[d, (k, b, s)] = hb[b, chunk_k_start + s, 128*core + d]
      "wvt" [128, HID] bf16: wvt[d, o] = Wv[o, 128*core+d]
      "bvS" [1, HID] bf16: bv on core 0, zeros elsewhere
      "consts" [1, 2] bf16: ones (bias-matmul stationary)
    Output "zout" [B, HID] f32: this core's contraction-partial of the
    unique output row (bias included on core 0, 1/S applied)."""
    nc = _new_nc()
    hbt = nc.dram_tensor("hbt", [128, B, 2, SH], BF16, kind="ExternalInput").ap()
    wvt = nc.dram_tensor("wvt", [128, HID], BF16, kind="ExternalInput").ap()
    # bvS[0, 0:2] = ones (bias-matmul stationary), bvS[0, 2:] = bv or 0
    bvS = nc.dram_tensor("bvS", [1, 2 + HID], BF16, kind="ExternalInput").ap()
    zout = nc.dram_tensor("zout", [B, HID], BF16, kind="ExternalOutput").ap()

    with tile.TileContext(nc) as tc:
        with (
            tc.tile_pool(name="big", bufs=1) as big,
            tc.tile_pool(name="small", bufs=1) as small,
            tc.tile_pool(name="psum", bufs=1, space="PSUM") as psum,
        ):
            hb_sb = big.tile([128, B, 2, SH], BF16)
            wvt_sb = big.tile([128, HID], BF16)
            bvS_sb = small.tile([1, 2 + HID], BF16)

            # Four 256KB pieces (batch x seq-half), two per HWDGE ring,
            # reduced by the DVE in arrival order.  The tiny bias row
            # rides SWDGE (off the HWDGE rings); wvt goes last on the
            # sync ring -- it gates only the final projection, which also
            # needs m (ready ~1.5us after the last hbt piece).
            nc.gpsimd.dma_start(bvS_sb[:], bvS[:])
            nc.sync.dma_start(hb_sb[:, 0, 0], hbt[:, 0, 0])
            nc.scalar.dma_start(hb_sb[:, 1, 0], hbt[:, 1, 0])
            nc.sync.dma_start(hb_sb[:, 0, 1], hbt[:, 0, 1])
            nc.scalar.dma_start(hb_sb[:, 1, 1], hbt[:, 1, 1])
            nc.sync.dma_start(wvt_sb[:], wvt[:])

            ones2 = bvS_sb[0:1, 0:2]
            scratch = small.tile([128, 512], BF16)
            nc.vector.memset(scratch[:], 1.0)
            # PE and DVE clock-gate warmups while the first DMAs stream
            pw = psum.tile([2, 512], F32, name="pwarm", tag="pwarm")
            for _ in range(6):
                nc.tensor.matmul(
                    pw[:], lhsT=scratch[:, 0:2], rhs=scratch[:], start=True, stop=True
                )
            # seq reduction: one [128, SH] free-axis DVE reduce per
            # (batch, half) in arrival order, a tiny combine, then the
            # exact 1/S mean scaling into the projection's bf16 stationary
            mtmp = small.tile([128, B * 2], BF16)
            mcmb = small.tile([128, B], BF16)
            m_sb = small.tile([128, B], BF16)
            with nc.allow_low_precision(reason="bf16 in, f32 accumulate"):
                for half in range(2):
                    for b2 in range(B):
                        nc.vector.reduce_sum(
                            mtmp[:, b2 * 2 + half : b2 * 2 + half + 1],
                            hb_sb[:, b2, half],
                            axis=mybir.AxisListType.X,
                        )
                nc.vector.reduce_sum(
                    mcmb[:],
                    mtmp[:].rearrange("p (b h) -> p b h", b=B),
                    axis=mybir.AxisListType.X,
                )
                nc.vector.tensor_scalar_mul(m_sb[:], mcmb[:], 1.0 / S)
            # keep the PE clock ungated between the warmups and the
            # projection, fed by batch-0's first piece
            for _ in range(2):
                nc.tensor.matmul(
                    pw[:],
                    lhsT=hb_sb[:, 0, 0, 0:2],
                    rhs=hb_sb[:, 0, 0, 0:512],
                    start=True,
                    stop=True,
                )

            # projection: z[b, o] = sum_d m[d, b]*wvt[d, o] + bv[o]; the
            # rank-1 bias matmul starts each accumulation group (it
            # depends only on the tiny early loads)
            z_sb = small.tile([B, HID], BF16)
            for h, weng in ((0, nc.sync), (1, nc.scalar)):
                zp = psum.tile([B, 512], F32, name=f"z{h}", tag=f"z{h}")
                nc.tensor.matmul(
                    zp[:],
                    lhsT=ones2,
                    rhs=bvS_sb[:, 2 + h * 512 : 2 + (h + 1) * 512],
                    start=True,
                    stop=False,
                )
                nc.tensor.matmul(
                    zp[:],
                    lhsT=m_sb[:],
                    rhs=wvt_sb[:, h * 512 : (h + 1) * 512],
                    start=False,
                    stop=True,
                )
                # plain evac (m carried the 1/S, bias is in the PSUM);
                # each half's write leaves on its own HWDGE ring as soon
                # as its evac lands
                with nc.allow_low_precision(reason="bf16 partials, host sums f32"):
                    nc.vector.tensor_copy(z_sb[:, h * 512 : (h + 1) * 512], zp[:])
                weng.dma_start(
                    zout[:, h * 512 : (h + 1) * 512], z_sb[:, h * 512 : (h + 1) * 512]
                )
    nc.compile()
    return nc


def get_nc():
    global _compiled
    if _compiled is None:
        _compiled = _build()
    return _compiled


def make_in_maps(inputs):
    hb = np.asarray(inputs["hidden_states_b"], dtype=np.float32)
    Wv = np.asarray(inputs["Wv"], dtype=np.float32)
    bv = np.asarray(inputs["bv"], dtype=np.float32)
    bvS = np.zeros((N_CORES, 1, 2 + HID), dtype=NPBF16)
    bvS[:, 0, 0:2] = 1  # bias-matmul stationary ones
    bvS[0, 0, 2:] = bv.astype(NPBF16)
    maps = []
    for c in range(N_CORES):
        sl = hb[:, :, c * D_LOC : (c + 1) * D_LOC].astype(NPBF16)  # [B, S, 128]
        # hbt[d, b, half, s] = hb[b, half*SH + s, c*128 + d]
        t = sl.reshape(B, 2, SH, D_LOC).transpose(3, 0, 1, 2)
        wt = Wv[:, c * D_LOC : (c + 1) * D_LOC].T.astype(NPBF16)  # [128 d, HID o]
        maps.append(
            {
                "hbt": np.ascontiguousarray(t),
                "wvt": np.ascontiguousarray(wt),
                "bvS": bvS[c],
            }
        )
    return maps


def combine(results):
    # unshard for contraction sharding: sum the 8 partials (bias was
    # folded into core 0's partial, 1/S scaling done on-device), then
    # broadcast the unique per-batch row over the sequence axis
    z = results[0]["zout"].astype(np.float32)
    for c in range(1, N_CORES):
        z += results[c]["zout"].astype(np.float32)
    return np.ascontiguousarray(np.broadcast_to(z[:, None, :], (B, S, HID)))


def kernel(**inputs) -> np.ndarray:
    nc = get_nc()
    res = run_bass_kernel_spmd(nc, make_in_maps(inputs), list(range(N_CORES)))
    return combine(res.results)


# revision 16
# speedup vs baseline: 1.2991x; 1.0045x over previous
"""Bass/Trainium2 kernel for nn_Differential_Attention_60825326846200.

Mathematical reduction of the reference:
  scores[b,h,i,j] = (sum_d q[b,h,i,d] - k[b,h,i,d]) / sqrt(DH) + mask[b,i]
is constant over the key index j, so the softmax over j is exactly the
uniform distribution (1/S) regardless of q, k, and the mask.  Hence
  ctx[b,h,i,:] = mean_j v[b,h,j,:]          (independent of i)
  out[b,i,:]   = (mean_j hidden_b[b,j,:]) @ Wv.T + bv   for every i.
The q/k projections and the attention mask cancel exactly, and the output
is rank-1 along the sequence axis: 2048 identical rows per batch.

ONE SPMD launch, contraction-sharded (no cross-core exchange needed):
core c owns HID columns d in [128c, 128c+128).

  Because the hidden dim (not the sequence) is sharded, each core's
  sequence reduction is COMPLETE for its slice: it reads
  hidden_b[:, :, d_c], reduces over all 2048 positions on the DVE
  (partitions = the 128 hidden columns, so m[d, b] lands already
  transposed for the projection lhsT), then contracts its 128 columns
  with its Wv slice -> z_c[b, o], a contraction-partial of the unique
  output row.  Core 0's bias input carries bv (others zeros), added via
  a rank-1 PE matmul into the same PSUM accumulation; m is pre-scaled
  by the exact 1/S so the PSUM holds final values and the evacuation is
  a plain copy.

  Host unshard = the standard gather for contraction sharding: sum the
  8 partials [2, 1024] and broadcast over the sequence axis (the output
  is rank-1: every row within a batch is the same vector).

  The inputs stream in bf16 (cast on the host while laying out the
  shards -- the 2e-2 tolerance is far above bf16's ~5e-3 here, and the
  f32 baseline already ran its matmuls in TF32-width float32r): 1.26MB
  in / 8KB out per core.  HW lessons baked in: HWDGE read bandwidth is
  ~130GB/s per ring on 4KB runs and the two rings share the 16 SDMA
  engines (~260GB/s aggregate); SWDGE (gpsimd) is ~50GB/s -- good only
  for tiny/small loads; partition-sliced DMAs use only half the SDMA
  engines (never split below 128 partitions); the first ACT op would
  emit a ~1.3us ACT_TABLE_LOAD ahead of the scalar ring's DMA issues,
  so no ACT ops are used at all.
"""

import numpy as np
import ml_dtypes

import concourse.bacc as bacc
import concourse.mybir as mybir
import concourse.tile as tile
from concourse.bass_utils import run_bass_kernel_spmd

N_CORES = 8
B, S, HID = 2, 2048, 1024
D_LOC = HID // N_CORES  # 128 hidden columns owned per core
SH = S // 2  # seq halves: 4 stream pieces (batch x half), reduced as they land
F32 = mybir.dt.float32
F32R = mybir.dt.float32r
BF16 = mybir.dt.bfloat16
NPBF16 = ml_dtypes.bfloat16

_compiled = None


def _new_nc():
    return bacc.Bacc(
        "TRN2",
        target_bir_lowering=False,
        debug=False,
        enable_asserts=False,
        num_devices=N_CORES,
    )


def _build():
    """Single launch: complete seq-reduction of this core's column slice,
    projection through its Wv rows, contraction-partial out.
    Inputs:
      "hbt" [128, sum(2*scs)] bf16, chunk-major flat cols (k, b, s):
        hbt[d, (k, b, s)] = hb[b, chunk_k_start + s, 128*core + d]
      "wvt" [128, HID] bf16: wvt[d, o] = Wv[o, 128*core+d]
      "bvS" [1, HID] bf16: bv on core 0, zeros elsewhere
      "consts" [1, 2] bf16: ones (bias-matmul stationary)
    Output "zout" [B, HID] f32: this core's contraction-partial of the
    unique output row (bias included on core 0, 1/S applied)."""
    nc = _new_nc()
    hbt = nc.dram_tensor("hbt", [128, B, 2, SH], BF16, kind="ExternalInput").ap()
    wvt = nc.dram_tensor("wvt", [128, HID], BF16, kind="ExternalInput").ap()
    # bvS[0, 0:2] = ones (bias-matmul stationary), bvS[0, 2:] = bv or 0
    bvS = nc.dram_tensor("bvS", [1, 2 + HID], BF16, kind="ExternalInput").ap()
    zout = nc.dram_tensor("zout", [B, HID], BF16, kind="ExternalOutput").ap()

    with tile.TileContext(nc) as tc:
        with (
            tc.tile_pool(name="big", bufs=1) as big,
            tc.tile_pool(name="small", bufs=1) as small,
            tc.tile_pool(name="psum", bufs=1, space="PSUM") as psum,
        ):
            hb_sb = big.tile([128, B, 2, SH], BF16)
            wvt_sb = big.tile([128, HID], BF16)
            bvS_sb = small.tile([1, 2 + HID], BF16)

            # Four 256KB pieces (batch x seq-half), two per HWDGE ring,
            # reduced by the DVE in arrival order.  The tiny bias row
            # rides SWDGE (off the HWDGE rings); wvt goes last on the
            # sync ring -- it gates only the final projection, which also
            # needs m (ready ~1.5us after the last hbt piece).
            nc.gpsimd.dma_start(bvS_sb[:], bvS[:])
            nc.sync.dma_start(hb_sb[:, 0, 0], hbt[:, 0, 0])
            nc.scalar.dma_start(hb_sb[:, 1, 0], hbt[:, 1, 0])
            nc.sync.dma_start(hb_sb[:, 0, 1], hbt[:, 0, 1])
            nc.scalar.dma_start(hb_sb[:, 1, 1], hbt[:, 1, 1])
            # wvt halves ride one ring each (balances the rings at 640KB;
            # each half feeds its own projection half)
            nc.sync.dma_start(wvt_sb[:, 0:512], wvt[:, 0:512])
            nc.scalar.dma_start(wvt_sb[:, 512:1024], wvt[:, 512:1024])

            ones2 = bvS_sb[0:1, 0:2]
            scratch = small.tile([128, 512], BF16)
            nc.vector.memset(scratch[:], 1.0)
            # PE and DVE clock-gate warmups while the first DMAs stream
            pw = psum.tile([2, 512], F32, name="pwarm", tag="pwarm")
            for _ in range(6):
                nc.tensor.matmul(
                    pw[:], lhsT=scratch[:, 0:2], rhs=scratch[:], start=True, stop=True
                )
            # seq reduction: one [128, SH] free-axis DVE reduce per
            # (batch, half) in arrival order, a tiny combine, then the
            # exact 1/S mean scaling into the projection's bf16 stationary
            # GpSimd (idle after its one DMA issue) tree-folds the last
            # piece 1024 -> 512 -> 256 so the DVE's final reduce is 4x
            # shorter; everything else reduces on the DVE in arrival order
            fold1 = small.tile([128, 512], BF16)
            fold2 = small.tile([128, 256], BF16)
            with nc.allow_low_precision(reason="bf16 partial-sum folds"):
                nc.gpsimd.tensor_add(
                    fold1[:], hb_sb[:, 1, 1, 0:512], hb_sb[:, 1, 1, 512:1024]
                )
                nc.gpsimd.tensor_add(fold2[:], fold1[:, 0:256], fold1[:, 256:512])
            mtmp = small.tile([128, B * 2], BF16)
            mcmb = small.tile([128, B], BF16)
            m_sb = small.tile([128, B], BF16)
            with nc.allow_low_precision(reason="bf16 in, f32 accumulate"):
                for b2, half in ((0, 0), (1, 0), (0, 1)):
                    nc.vector.reduce_sum(
                        mtmp[:, b2 * 2 + half : b2 * 2 + half + 1],
                        hb_sb[:, b2, half],
                        axis=mybir.AxisListType.X,
                    )
                nc.vector.reduce_sum(
                    mtmp[:, 3:4], fold2[:], axis=mybir.AxisListType.X
                )
                nc.vector.reduce_sum(
                    mcmb[:],
                    mtmp[:].rearrange("p (b h) -> p b h", b=B),
                    axis=mybir.AxisListType.X,
                )
                nc.vector.tensor_scalar_mul(m_sb[:], mcmb[:], 1.0 / S)
            # keep the PE clock ungated between the warmups and the
            # projection, fed by batch-0's first piece
            for _ in range(2):
                nc.tensor.matmul(
                    pw[:],
                    lhsT=hb_sb[:, 0, 0, 0:2],
                    rhs=hb_sb[:, 0, 0, 0:512],
                    start=True,
                    stop=True,
                )

            # projection: z[b, o] = sum_d m[d, b]*wvt[d, o] + bv[o]; the
            # rank-1 bias matmul starts each accumulation group (it
            # depends only on the tiny early loads)
            z_sb = small.tile([B, HID], BF16)
            for h, weng in ((0, nc.sync), (1, nc.scalar)):
                zp = psum.tile([B, 512], F32, name=f"z{h}", tag=f"z{h}")
                nc.tensor.matmul(
                    zp[:],
                    lhsT=ones2,
                    rhs=bvS_sb[:, 2 + h * 512 : 2 + (h + 1) * 512],
                    start=True,
                    stop=False,
                )
                nc.tensor.matmul(
                    zp[:],
                    lhsT=m_sb[:],
                    rhs=wvt_sb[:, h * 512 : (h + 1) * 512],
                    start=False,
                    stop=True,
                )
                # plain evac (m carried the 1/S, bias is in the PSUM);
                # each half's write leaves on its own HWDGE ring as soon
                # as its evac lands
                with nc.allow_low_precision(reason="bf16 partials, host sums f32"):
                    nc.vector.tensor_copy(z_sb[:, h * 512 : (h + 1) * 512], zp[:])
                weng.dma_start(
                    zout[:, h * 512 : (h + 1) * 512], z_sb[:, h * 512 : (h + 1) * 512]
                )
    nc.compile()
    return nc


def get_nc():
    global _compiled
    if _compiled is None:
        _compiled = _build()
    return _compiled


def make_in_maps(inputs):
    hb = np.asarray(inputs["hidden_states_b"], dtype=np.float32)
    Wv = np.asarray(inputs["Wv"], dtype=np.float32)
    bv = np.asarray(inputs["bv"], dtype=np.float32)
    bvS = np.zeros((N_CORES, 1, 2 + HID), dtype=NPBF16)
    bvS[:, 0, 0:2] = 1  # bias-matmul stationary ones
    bvS[0, 0, 2:] = bv.astype(NPBF16)
    maps = []
    for c in range(N_CORES):
        sl = hb[:, :, c * D_LOC : (c + 1) * D_LOC].astype(NPBF16)  # [B, S, 128]
        # hbt[d, b, half, s] = hb[b, half*SH + s, c*128 + d]
        t = sl.reshape(B, 2, SH, D_LOC).transpose(3, 0, 1, 2)
        wt = Wv[:, c * D_LOC : (c + 1) * D_LOC].T.astype(NPBF16)  # [128 d, HID o]
        maps.append(
            {
                "hbt": np.ascontiguousarray(t),
                "wvt": np.ascontiguousarray(wt),
                "bvS": bvS[c],
            }
        )
    return maps


def combine(results):
    # unshard for contraction sharding: sum the 8 partials (bias was
    # folded into core 0's partial, 1/S scaling done on-device), then
    # broadcast the unique per-batch row over the sequence axis
    z = results[0]["zout"].astype(np.float32)
    for c in range(1, N_CORES):
        z += results[c]["zout"].astype(np.float32)
    return np.ascontiguousarray(np.broadcast_to(z[:, None, :], (B, S, HID)))


def kernel(**inputs) -> np.ndarray:
    nc = get_nc()
    res = run_bass_kernel_spmd(nc, make_in_maps(inputs), list(range(N_CORES)))
    return combine(res.results)
